# revision 4
# baseline (speedup 1.0000x reference)
"""LoRA embedding lookup kernel for 8 TRN2 NeuronCores.

Computes out[b,s,:] = lora_down[input_ids[b,s], :] @ lora_up  (f32).

Sharding: data-parallel over tokens. 8192 tokens are split into 8 shards of
1024 tokens; each core gathers its rows of lora_down (indirect DMA), PE-
transposes them to [rank, tokens], matmuls against lora_up (replicated), and
writes its [1024, 4096] output shard. No collectives needed; the host
concatenates the shards.

Shapes are hardcoded per the problem spec:
  input_ids [4, 2048] int  |  lora_down [32000, 16] f32  |  lora_up [16, 4096] f32
"""

import numpy as np

import concourse.bacc as bacc
import concourse.bass as bass
import concourse.mybir as mybir
import concourse.tile as tile
from concourse.bass_utils import run_bass_kernel_spmd
from concourse.masks import make_identity

VOCAB = 32000
RANK = 16
DIM = 4096
N_CORES = 8
P = 128
DIM_CHUNK = 512

MM_MODE = "fp32"  # fp32 | fp32r


def build_nc(tokens_per_core: int, mm_mode: str = MM_MODE) -> bass.Bass:
    n_tok_tiles = tokens_per_core // P
    n_dim_chunks = DIM // DIM_CHUNK

    nc = bacc.Bacc(trn_type="TRN2", target_bir_lowering=False, debug=False)
    ids_d = nc.dram_tensor(
        "ids", [P, n_tok_tiles], mybir.dt.int32, kind="ExternalInput"
    ).ap()
    table_d = nc.dram_tensor(
        "table", [VOCAB, RANK], mybir.dt.float32, kind="ExternalInput"
    ).ap()
    up_d = nc.dram_tensor(
        "up", [RANK, DIM], mybir.dt.float32, kind="ExternalInput"
    ).ap()
    out_d = nc.dram_tensor(
        "out", [tokens_per_core, DIM], mybir.dt.float32, kind="ExternalOutput"
    ).ap()

    with tile.TileContext(nc) as tc:
        with (
            tc.tile_pool(name="const", bufs=1) as cpool,
            tc.tile_pool(name="gather", bufs=4) as gpool,
            tc.tile_pool(name="gtp", bufs=n_tok_tiles) as gtpool,
            tc.tile_pool(name="outp", bufs=3) as opool,
            tc.tile_pool(name="pst", bufs=2, space="PSUM") as ptpool,
            tc.tile_pool(name="psm", bufs=6, space="PSUM") as pmpool,
        ):
            identity = cpool.tile([P, P], mybir.dt.float32)
            make_identity(nc, identity[:])
            u = cpool.tile([RANK, DIM], mybir.dt.float32)
            nc.sync.dma_start(out=u[:], in_=up_d[:, :])
            ids = cpool.tile([P, n_tok_tiles], mybir.dt.int32)
            nc.sync.dma_start(out=ids[:], in_=ids_d[:, :])

            # Gather + transpose: G_c [128 tokens, 16] -> GT_c [16, 128]
            gts = []
            for c in range(n_tok_tiles):
                g = gpool.tile([P, RANK], mybir.dt.float32, tag="g")
                nc.gpsimd.indirect_dma_start(
                    out=g[:],
                    out_offset=None,
                    in_=table_d[:, :],
                    in_offset=bass.IndirectOffsetOnAxis(ap=ids[:, c : c + 1], axis=0),
                )
                gpsum = ptpool.tile([RANK, P], mybir.dt.float32, tag="gpsum")
                nc.tensor.transpose(out=gpsum[:], in_=g[:], identity=identity[:])
                gt = gtpool.tile([RANK, P], mybir.dt.float32, tag="gt")
                nc.vector.tensor_copy(out=gt[:], in_=gpsum[:])
                gts.append(gt)

            # Matmul: out[tok_tile] = GT_c.T @ U, chunked over dim
            for c in range(n_tok_tiles):
                o = opool.tile([P, DIM], mybir.dt.float32, tag="o")
                for n in range(n_dim_chunks):
                    lhsT = gts[c][:]
                    rhs = u[:, n * DIM_CHUNK : (n + 1) * DIM_CHUNK]
                    if mm_mode == "fp32r":
                        lhsT = lhsT.bitcast(mybir.dt.float32r)
                        rhs = rhs.bitcast(mybir.dt.float32r)
                    ps = pmpool.tile([P, DIM_CHUNK], mybir.dt.float32, tag="ps")
                    nc.tensor.matmul(out=ps[:], lhsT=lhsT, rhs=rhs, start=True, stop=True)
                    nc.vector.tensor_copy(
                        out=o[:, n * DIM_CHUNK : (n + 1) * DIM_CHUNK], in_=ps[:]
                    )
                nc.sync.dma_start(out=out_d[c * P : (c + 1) * P, :], in_=o[:])
    nc.compile()
    return nc


def _kernel_impl(input_ids, lora_down, lora_up, mm_mode=MM_MODE, trace=False):
    input_ids = np.asarray(input_ids)
    lora_down = np.ascontiguousarray(np.asarray(lora_down, dtype=np.float32))
    lora_up = np.ascontiguousarray(np.asarray(lora_up, dtype=np.float32))
    B, S = input_ids.shape
    total = B * S
    tokens_per_core = total // N_CORES
    n_tok_tiles = tokens_per_core // P

    ids_flat = input_ids.reshape(-1).astype(np.int32)
    nc = build_nc(tokens_per_core, mm_mode)
    in_maps = []
    for i in range(N_CORES):
        shard = ids_flat[i * tokens_per_core : (i + 1) * tokens_per_core]
        ids_t = np.ascontiguousarray(shard.reshape(n_tok_tiles, P).T)  # [128, n_tiles]
        in_maps.append({"ids": ids_t, "table": lora_down, "up": lora_up})
    res = run_bass_kernel_spmd(
        nc, in_maps, core_ids=list(range(N_CORES)), trace=trace
    )
    shards = [np.asarray(res.results[i]["out"]) for i in range(N_CORES)]
    out = np.concatenate(shards, axis=0).reshape(B, S, DIM)
    return np.ascontiguousarray(out.astype(np.float32)), res


def kernel(input_ids, lora_down, lora_up):
    out, _ = _kernel_impl(input_ids, lora_down, lora_up)
    return out
